# revision 5
# baseline (speedup 1.0000x reference)
"""Trainium2 Bass kernel for (W0 (x) W1 (x) W2 (x) W3) @ x  -- Kronecker chain.

Shapes: x [2^20, 32] fp32, Wi [32, 32] fp32. Output [2^20, 32] fp32.

Strategy (8 NeuronCores, batch-sharded: core c owns x[:, 4c:4c+4]):
View x_c as t[j0,j1,j2,j3,b] = [32,32,32,32,4]. Four 32-contractions.

PE-crossing architecture: stages 1-3 run the matmul with the DATA as the
stationary operand and an augmented 128x128 weight (delta-structured
W (x) I4 permutation) as the moving operand:
    out[m, n] = sum_k D[k, m] * Waug[k, n]
so the output partitions become the chunk's columns -- the NEXT contraction
mode rotates onto the partition axis inside the matmul itself. Evacuations
(PSUM -> SBUF, fp32 -> bf16) are flat copies, alternated between the Scalar
and Vector engines. Stage 4 is a normal weight-stationary matmul whose
output partition order (i0*4 + i3b) makes the store contiguous.

Stage order: j2, j3, j1, j0. Transient partition spectators: j1h -> b -> i2b
-> i3b. Phase I streams 32 chunks over j0 (load + S1 + S2 -> B1 bf16
resident, 64KB/partition). Phase II streams 8 chunks over i2a
(S3 + S4 + store). Host pre-shuffles x (bf16 cast) and post-shuffles y so
every DMA is a full-bandwidth contiguous transfer.
"""
import numpy as np
import ml_dtypes

import concourse.bass as bass
import concourse.bacc as bacc
import concourse.mybir as mybir
import concourse.tile as tile
from concourse.bass_utils import run_bass_kernel_spmd

F32 = mybir.dt.float32
BF16 = mybir.dt.bfloat16

L = 32
N = L ** 4          # 1048576
B = 32
NCORES = 8
BC = B // NCORES    # 4

_NC_CACHE = {}


def _build_nc():
    nc = bacc.Bacc("TRN2", target_bir_lowering=False, debug=False)

    # x pre-shuffled on host to [j0, (j1h, j2), (j1l, j3, b)], bf16
    x = nc.dram_tensor("x", [32, 131072], BF16, kind="ExternalInput")
    w2a = nc.dram_tensor("w2a", [128, 128], BF16, kind="ExternalInput")
    w3a = nc.dram_tensor("w3a", [128, 128], BF16, kind="ExternalInput")
    w1a = nc.dram_tensor("w1a", [128, 128], BF16, kind="ExternalInput")
    w0a = nc.dram_tensor("w0a", [128, 128], BF16, kind="ExternalInput")
    # y device order: [i2a(8), (i0, i3b)(128), (i3a, b, i1, i2b)(4096)] fp32
    y = nc.dram_tensor("y", [8, 524288], BF16, kind="ExternalOutput")

    # Greedy least-loaded PSUM->SBUF evacuation across Act/DVE/Pool.
    # Costs mirror the TRN2 cost model (ns): Act (rows+222)/1.2,
    # DVE (rows+120)/0.96, Pool 95 + rows/(1.2*0.6).
    busy = {"scalar": 0.0, "vector": 0.0, "gpsimd": 0.0}

    def _evac_cost(eng, rows):
        if eng == "scalar":
            return (rows + 222) / 1.2
        if eng == "vector":
            return (rows + 120) / 0.96
        return 95.0 + rows / 0.72

    def evac(idx, out_ap, in_ap, rows=1024, force=None):
        eng = force or min(busy, key=lambda e: busy[e] + _evac_cost(e, rows))
        busy[eng] += _evac_cost(eng, rows)
        if eng == "scalar":
            nc.scalar.copy(out=out_ap, in_=in_ap)
        elif eng == "vector":
            nc.vector.tensor_copy(out_ap, in_ap)
        else:
            nc.gpsimd.tensor_copy(out_ap, in_ap)

    with tile.TileContext(nc) as tc:
        with tc.tile_pool(name="wp", bufs=1) as wp, \
             tc.tile_pool(name="b1p", bufs=1) as b1p:
            w2s = wp.tile([128, 128], BF16, name="w2s")
            w3s = wp.tile([128, 128], BF16, name="w3s")
            w1s = wp.tile([128, 128], BF16, name="w1s")
            w0s = wp.tile([128, 128], BF16, name="w0s")
            # w2 + first data chunk go first so S1 can start ASAP; the
            # remaining weights trickle in behind the first few loads.
            nc.sync.dma_start(out=w2s[:], in_=w2a.ap())

            # B1: [part (i2b,j1), addr = i2a*4096 + i3a*512 + b*128 + i3b*32 + j0] bf16
            b1 = b1p.tile([128, 32768], BF16, name="b1")
            b1_t, b1_o = b1.tensor, b1.offset

            # ---- Phase I: S1 (contract j2) + S2 (contract j3), per j0 ----
            with tc.tile_pool(name="lp", bufs=4) as lp, \
                 tc.tile_pool(name="t1p", bufs=4) as t1p, \
                 tc.tile_pool(name="ps1", bufs=2, space="PSUM") as ps1, \
                 tc.tile_pool(name="ps2", bufs=2, space="PSUM") as ps2:
                for j0 in range(32):
                    lt = lp.tile([128, 1024], BF16, name="lt")
                    lt_t, lt_o = lt.tensor, lt.offset
                    nc.sync.dma_start(
                        out=lt[:],
                        in_=bass.AP(x, j0 * 131072, [[1024, 128], [1, 1024]]))
                    if j0 == 0:
                        nc.sync.dma_start(out=w3s[:], in_=w3a.ap())
                    elif j0 == 1:
                        nc.sync.dma_start(out=w1s[:], in_=w1a.ap())
                    elif j0 == 2:
                        nc.sync.dma_start(out=w0s[:], in_=w0a.ap())

                    # T1: [part (j3, b), free addr = i2a*128+i2b*32+j1h*8+j1l]
                    t1 = t1p.tile([128, 1024], BF16, name="t1")
                    t1_t, t1_o = t1.tensor, t1.offset
                    p1 = ps1.tile([128, 1024], F32, name="p1")
                    for j1l in range(8):
                        lhsT = bass.AP(lt_t, lt_o + j1l * 128,
                                       [[1024, 128], [1, 128]])
                        nc.tensor.matmul(p1[:, j1l * 128:(j1l + 1) * 128],
                                         lhsT, w2s[:], start=True, stop=True)
                    # psum pos (j1l, n1=(i2a,i2b,j1h)); merge (i2b,j1h)->[8,16]
                    outap = bass.AP(t1_t, t1_o,
                                    [[1024, 128], [1, 8], [128, 8], [8, 16]])
                    evac(j0, outap, p1[:])

                    p2 = ps2.tile([128, 1024], F32, name="p2")
                    for i2a in range(8):
                        lhsT = bass.AP(t1_t, t1_o + i2a * 128,
                                       [[1024, 128], [1, 128]])
                        nc.tensor.matmul(p2[:, i2a * 128:(i2a + 1) * 128],
                                         lhsT, w3s[:], start=True, stop=True)
                    # psum pos (i2a, n2=(i3a,b,i3b)); merge (b,i3b)->[32,16]
                    outap = bass.AP(b1_t, b1_o + j0,
                                    [[32768, 128], [4096, 8], [512, 8], [32, 16]])
                    evac(j0 + 1, outap, p2[:])

            # ---- Phase II: S3 (contract j1) + S4 (contract j0), per i2a ----
            with tc.tile_pool(name="t3p", bufs=3) as t3p, \
                 tc.tile_pool(name="stgp", bufs=3) as stgp, \
                 tc.tile_pool(name="ps3", bufs=2, space="PSUM") as ps3, \
                 tc.tile_pool(name="ps4", bufs=2, space="PSUM") as ps4:
                for k in range(8):  # k = i2a
                    # T3: [part (i3b,j0), free (i3a:512, b:128, (i1*4+i2b):1)]
                    t3 = t3p.tile([128, 4096], BF16, name="t3")
                    t3_t, t3_o = t3.tensor, t3.offset
                    for th in range(4):  # pairs of i3a
                        p3 = ps3.tile([128, 1024], F32, name="p3")
                        for q in range(8):
                            cq = 8 * th + q      # cq = i3a*4 + b
                            lhsT = bass.AP(b1_t,
                                           b1_o + k * 4096 + cq * 128,
                                           [[32768, 128], [1, 128]])
                            nc.tensor.matmul(p3[:, q * 128:(q + 1) * 128],
                                             lhsT, w1s[:], start=True, stop=True)
                        evac(th, t3[:, th * 1024:(th + 1) * 1024], p3[:])

                    stg = stgp.tile([128, 4096], BF16, name="stg")
                    stg_t, stg_o = stg.tensor, stg.offset
                    for th in range(4):  # pairs of i3a
                        p4 = ps4.tile([128, 1024], F32, name="p4")
                        for m in range(2):
                            i3a = 2 * th + m
                            rhs = bass.AP(t3_t, t3_o + i3a * 512,
                                          [[4096, 128], [128, 4], [1, 128]])
                            nc.tensor.matmul(p4[:, m * 512:(m + 1) * 512],
                                             w0s[:], rhs, start=True, stop=True)
                        evac(th + 1, stg[:, th * 1024:(th + 1) * 1024], p4[:])
                        if k == 7:
                            # tail: stream the final chunk out in quarters so
                            # the last store doesn't serialize after all evacs
                            nc.sync.dma_start(
                                out=bass.AP(y, k * 524288 + th * 1024,
                                            [[4096, 128], [1, 1024]]),
                                in_=bass.AP(stg_t, stg_o + th * 1024,
                                            [[4096, 128], [1, 1024]]))

                    if k < 7:
                        nc.sync.dma_start(
                            out=bass.AP(y, k * 524288, [[4096, 128], [1, 4096]]),
                            in_=bass.AP(stg_t, stg_o, [[4096, 128], [1, 4096]]))

    nc.finalize()
    return nc


def _build_waug(w: np.ndarray, kind: str) -> np.ndarray:
    """Augmented 128x128 weights (see apsim2.py)."""
    wa = np.zeros((128, 128), dtype=np.float32)
    ar = np.arange(32)
    if kind == "w3":
        # rows p = j3*4 + b ; cols n = i3a*16 + b*4 + i3b
        for b in range(4):
            cols = (ar >> 2) * 16 + b * 4 + (ar & 3)
            wa[np.ix_(ar * 4 + b, cols)] = w.T
    else:
        # rows p = q*32 + j ; cols n = i*4 + q
        for q in range(4):
            wa[np.ix_(q * 32 + ar, ar * 4 + q)] = w.T
    return wa


def _get_nc():
    if "nc" not in _NC_CACHE:
        _NC_CACHE["nc"] = _build_nc()
    return _NC_CACHE["nc"]


def make_in_maps(x, W0, W1, W2, W3):
    x = np.asarray(x, dtype=np.float32)
    bf = ml_dtypes.bfloat16
    w2a = _build_waug(np.asarray(W2, np.float32), "q").astype(bf)
    w3a = _build_waug(np.asarray(W3, np.float32), "w3").astype(bf)
    w1a = _build_waug(np.asarray(W1, np.float32), "q").astype(bf)
    w0a = _build_waug(np.asarray(W0, np.float32), "q").astype(bf)
    xr = x.reshape(32, 4, 8, 32, 32, B)
    in_maps = []
    for c in range(NCORES):
        xc = xr[..., c * BC:(c + 1) * BC].transpose(0, 1, 3, 2, 4, 5)
        xc = np.ascontiguousarray(xc).astype(bf).reshape(32, 131072)
        in_maps.append({"x": xc, "w2a": w2a, "w3a": w3a,
                        "w1a": w1a, "w0a": w0a})
    return in_maps


def _unshuffle_y(yd: np.ndarray) -> np.ndarray:
    """[i2a(8), (i0, i3b), (i3a, b, i1, i2b)] -> [N, BC]."""
    y = yd.astype(np.float32).reshape(8, 32, 4, 8, BC, 32, 4)
    y = y.transpose(1, 5, 0, 6, 3, 2, 4)
    return np.ascontiguousarray(y).reshape(N, BC)


def kernel(x, W0, W1, W2, W3, _trace=False):
    nc = _get_nc()
    in_maps = make_in_maps(x, W0, W1, W2, W3)
    res = run_bass_kernel_spmd(nc, in_maps, core_ids=list(range(NCORES)),
                               trace=_trace)
    out = np.concatenate(
        [_unshuffle_y(res.results[c]["y"]) for c in range(NCORES)], axis=1)
    if _trace:
        kernel.last_result = res
    return out


if __name__ == "__main__":
    rng = np.random.default_rng(0)
    x = rng.standard_normal((N, B), dtype=np.float32)
    ws = [rng.standard_normal((L, L), dtype=np.float32) for _ in range(4)]
    y = kernel(x, *ws)
    print("ran", y.shape, y.dtype)



# revision 6
# speedup vs baseline: 1.3195x; 1.3195x over previous
"""Trainium2 Bass kernel for (W0 (x) W1 (x) W2 (x) W3) @ x  -- Kronecker chain.

Shapes: x [2^20, 32] fp32, Wi [32, 32] fp32. Output [2^20, 32] fp32.

Strategy (8 NeuronCores, batch-sharded: core c owns x[:, 4c:4c+4]):
View x_c as t[j0,j1,j2,j3,b] = [32,32,32,32,4]. Four 32-contractions.

PE-crossing architecture: stages 1-3 run the matmul with the DATA as the
stationary operand and an augmented 128x128 weight (delta-structured
W (x) I4 permutation) as the moving operand:
    out[m, n] = sum_k D[k, m] * Waug[k, n]
so the output partitions become the chunk's columns -- the NEXT contraction
mode rotates onto the partition axis inside the matmul itself. Stage 4 is a
normal weight-stationary matmul whose output partition order (i0*4 + i3b)
makes the store contiguous.

Stage order: j2, j3, j1, j0. Phase I streams 32 chunks over j0 (load +
S1 + S2 -> B1 bf16 resident); Phase II streams 8 chunks over i2a
(S3 + S4 + store). Host pre-shuffles x (bf16 cast) and post-shuffles y.

Schedule: both phases are software-pipelined at half-chunk granularity.
PSUM is split into [128,512] tiles (1 bank) with 4 bufs per stage; the PE
program order interleaves stage A of micro-step u with stage B of
micro-step u-4, so each PSUM->SBUF evacuation has ~1.7us to complete
before the PE needs its result or its bank. Evacuations are assigned
greedily (least-loaded, exact cost model) across Act/DVE/Pool.
"""
import numpy as np
import ml_dtypes

import concourse.bass as bass
import concourse.bacc as bacc
import concourse.mybir as mybir
import concourse.tile as tile
from concourse.bass_utils import run_bass_kernel_spmd

F32 = mybir.dt.float32
BF16 = mybir.dt.bfloat16

L = 32
N = L ** 4          # 1048576
B = 32
NCORES = 8
BC = B // NCORES    # 4
SKEW = 4            # micro-steps between a stage's output and its consumer

_NC_CACHE = {}


def _build_nc():
    nc = bacc.Bacc("TRN2", target_bir_lowering=False, debug=False)

    # x pre-shuffled on host to [j0, (j1h, j2), (j1l, j3, b)], bf16
    x = nc.dram_tensor("x", [32, 131072], BF16, kind="ExternalInput")
    w2a = nc.dram_tensor("w2a", [128, 128], BF16, kind="ExternalInput")
    w3a = nc.dram_tensor("w3a", [128, 128], BF16, kind="ExternalInput")
    w1a = nc.dram_tensor("w1a", [128, 128], BF16, kind="ExternalInput")
    w0a = nc.dram_tensor("w0a", [128, 128], BF16, kind="ExternalInput")
    # y device order: [i2a(8), (i0, i3b)(128), (i3a, b, i1, i2b)(4096)] fp32
    y = nc.dram_tensor("y", [8, 524288], BF16, kind="ExternalOutput")

    # Greedy least-loaded PSUM->SBUF evacuation across Act/DVE/Pool.
    # Costs mirror the TRN2 cost model (ns): Act (rows+222)/1.2,
    # DVE (rows+120)/0.96, Pool 95 + rows/(1.2*0.6).
    busy = {"scalar": 0.0, "vector": 0.0, "gpsimd": 0.0}

    def _evac_cost(eng, rows):
        if eng == "scalar":
            return (rows + 222) / 1.2
        if eng == "vector":
            return (rows + 120) / 0.96
        return 95.0 + rows / 0.72

    def evac(out_ap, in_ap, rows=512, force=None):
        eng = force or min(busy, key=lambda e: busy[e] + _evac_cost(e, rows))
        busy[eng] += _evac_cost(eng, rows)
        if eng == "scalar":
            nc.scalar.copy(out=out_ap, in_=in_ap)
        elif eng == "vector":
            nc.vector.tensor_copy(out_ap, in_ap)
        else:
            nc.gpsimd.tensor_copy(out_ap, in_ap)

    with tile.TileContext(nc) as tc:
        with tc.tile_pool(name="wp", bufs=1) as wp, \
             tc.tile_pool(name="b1p", bufs=1) as b1p:
            w2s = wp.tile([128, 128], BF16, name="w2s")
            w3s = wp.tile([128, 128], BF16, name="w3s")
            w1s = wp.tile([128, 128], BF16, name="w1s")
            w0s = wp.tile([128, 128], BF16, name="w0s")

            # B1: [part (i2b,j1), addr = i2a*4096 + i3a*512 + b*128 + i3b*32 + j0]
            b1 = b1p.tile([128, 32768], BF16, name="b1")
            b1_t, b1_o = b1.tensor, b1.offset

            # ---- Phase I: S1 (contract j2) + S2 (contract j3) ----
            # micro-step u = 2*j0 + h, h in {0,1}: S1 does j1l in 4h..4h+3,
            # S2 (at u-SKEW) does i2a in 4h..4h+3.
            with tc.tile_pool(name="lp", bufs=4) as lp, \
                 tc.tile_pool(name="t1p", bufs=4) as t1p, \
                 tc.tile_pool(name="ps1", bufs=4, space="PSUM") as ps1, \
                 tc.tile_pool(name="ps2", bufs=4, space="PSUM") as ps2:

                lts = {}
                t1s = {}

                def load_chunk(c):
                    lt = lp.tile([128, 1024], BF16, name="lt")
                    lts[c] = (lt.tensor, lt.offset)
                    nc.sync.dma_start(
                        out=lt[:],
                        in_=bass.AP(x, c * 131072, [[1024, 128], [1, 1024]]))

                # startup: w2 + first chunk first, other weights behind
                nc.sync.dma_start(out=w2s[:], in_=w2a.ap())
                load_chunk(0)
                nc.sync.dma_start(out=w3s[:], in_=w3a.ap())
                load_chunk(1)
                nc.sync.dma_start(out=w1s[:], in_=w1a.ap())
                load_chunk(2)
                nc.sync.dma_start(out=w0s[:], in_=w0a.ap())

                for u in range(64 + SKEW):
                    if u < 64:
                        c, h = u // 2, u % 2
                        if h == 0:
                            if c + 3 < 32:
                                load_chunk(c + 3)
                            t1 = t1p.tile([128, 1024], BF16, name="t1")
                            t1s[c] = (t1.tensor, t1.offset)
                        lt_t, lt_o = lts[c]
                        t1_t, t1_o = t1s[c]
                        p1 = ps1.tile([128, 512], F32, name="p1")
                        for d in range(4):
                            j1l = 4 * h + d
                            lhsT = bass.AP(lt_t, lt_o + j1l * 128,
                                           [[1024, 128], [1, 128]])
                            nc.tensor.matmul(p1[:, d * 128:(d + 1) * 128],
                                             lhsT, w2s[:], start=True, stop=True)
                        # psum cols (j1l_lo, i2a, i2b, j1h) -> t1 addr
                        # i2a*128 + i2b*32 + j1h*8 + j1l
                        evac(bass.AP(t1_t, t1_o + 4 * h,
                                     [[1024, 128], [1, 4], [128, 8], [8, 16]]),
                             p1[:])

                    if u >= SKEW:
                        c2, h2 = (u - SKEW) // 2, (u - SKEW) % 2
                        t1_t, t1_o = t1s[c2]
                        p2 = ps2.tile([128, 512], F32, name="p2")
                        for d in range(4):
                            i2a = 4 * h2 + d
                            lhsT = bass.AP(t1_t, t1_o + i2a * 128,
                                           [[1024, 128], [1, 128]])
                            nc.tensor.matmul(p2[:, d * 128:(d + 1) * 128],
                                             lhsT, w3s[:], start=True, stop=True)
                        # psum cols (i2a_lo, i3a, b, i3b) -> b1 addr
                        # i2a*4096 + i3a*512 + b*128 + i3b*32 + j0
                        evac(bass.AP(b1_t, b1_o + c2 + 4 * h2 * 4096,
                                     [[32768, 128], [4096, 4], [512, 8], [32, 16]]),
                             p2[:])

            # ---- Phase II: S3 (contract j1) + S4 (contract j0) ----
            # micro-step v = 8*k + g: S3 does cq in 4g..4g+3 (cq=i3a*4+b),
            # S4 (at v-SKEW) does i3a = g.
            with tc.tile_pool(name="t3p", bufs=3) as t3p, \
                 tc.tile_pool(name="stgp", bufs=3) as stgp, \
                 tc.tile_pool(name="ps3", bufs=4, space="PSUM") as ps3, \
                 tc.tile_pool(name="ps4", bufs=4, space="PSUM") as ps4:

                t3s = {}
                stgs = {}
                for v in range(64 + SKEW):
                    if v < 64:
                        k, g = v // 8, v % 8
                        if g == 0:
                            t3 = t3p.tile([128, 4096], BF16, name="t3")
                            t3s[k] = (t3.tensor, t3.offset)
                            stg = stgp.tile([128, 4096], BF16, name="stg")
                            stgs[k] = (stg.tensor, stg.offset)
                        t3_t, t3_o = t3s[k]
                        p3 = ps3.tile([128, 512], F32, name="p3")
                        for d in range(4):
                            cq = 4 * g + d
                            lhsT = bass.AP(b1_t, b1_o + k * 4096 + cq * 128,
                                           [[32768, 128], [1, 128]])
                            nc.tensor.matmul(p3[:, d * 128:(d + 1) * 128],
                                             lhsT, w1s[:], start=True, stop=True)
                        # psum cols map flat into t3: addr = i3a*512 + b*128 + n
                        evac(bass.AP(t3_t, t3_o + g * 512,
                                     [[4096, 128], [1, 512]]),
                             p3[:])

                    if v >= SKEW:
                        k2, g2 = (v - SKEW) // 8, (v - SKEW) % 8
                        t3_t, t3_o = t3s[k2]
                        stg_t, stg_o = stgs[k2]
                        p4 = ps4.tile([128, 512], F32, name="p4")
                        rhs = bass.AP(t3_t, t3_o + g2 * 512,
                                      [[4096, 128], [128, 4], [1, 128]])
                        nc.tensor.matmul(p4[:], w0s[:], rhs,
                                         start=True, stop=True)
                        evac(bass.AP(stg_t, stg_o + g2 * 512,
                                     [[4096, 128], [1, 512]]),
                             p4[:])
                        if k2 < 7:
                            if g2 == 7:
                                nc.sync.dma_start(
                                    out=bass.AP(y, k2 * 524288,
                                                [[4096, 128], [1, 4096]]),
                                    in_=bass.AP(stg_t, stg_o,
                                                [[4096, 128], [1, 4096]]))
                        elif g2 % 2 == 1:
                            # tail: stream the final chunk out in quarters
                            nc.sync.dma_start(
                                out=bass.AP(y, k2 * 524288 + (g2 - 1) * 512,
                                            [[4096, 128], [1, 1024]]),
                                in_=bass.AP(stg_t, stg_o + (g2 - 1) * 512,
                                            [[4096, 128], [1, 1024]]))

    nc.finalize()
    return nc


def _build_waug(w: np.ndarray, kind: str) -> np.ndarray:
    """Augmented 128x128 weights (see apsim2.py)."""
    wa = np.zeros((128, 128), dtype=np.float32)
    ar = np.arange(32)
    if kind == "w3":
        # rows p = j3*4 + b ; cols n = i3a*16 + b*4 + i3b
        for b in range(4):
            cols = (ar >> 2) * 16 + b * 4 + (ar & 3)
            wa[np.ix_(ar * 4 + b, cols)] = w.T
    else:
        # rows p = q*32 + j ; cols n = i*4 + q
        for q in range(4):
            wa[np.ix_(q * 32 + ar, ar * 4 + q)] = w.T
    return wa


def _get_nc():
    if "nc" not in _NC_CACHE:
        _NC_CACHE["nc"] = _build_nc()
    return _NC_CACHE["nc"]


def make_in_maps(x, W0, W1, W2, W3):
    x = np.asarray(x, dtype=np.float32)
    bf = ml_dtypes.bfloat16
    w2a = _build_waug(np.asarray(W2, np.float32), "q").astype(bf)
    w3a = _build_waug(np.asarray(W3, np.float32), "w3").astype(bf)
    w1a = _build_waug(np.asarray(W1, np.float32), "q").astype(bf)
    w0a = _build_waug(np.asarray(W0, np.float32), "q").astype(bf)
    xr = x.reshape(32, 4, 8, 32, 32, B)
    in_maps = []
    for c in range(NCORES):
        xc = xr[..., c * BC:(c + 1) * BC].transpose(0, 1, 3, 2, 4, 5)
        xc = np.ascontiguousarray(xc).astype(bf).reshape(32, 131072)
        in_maps.append({"x": xc, "w2a": w2a, "w3a": w3a,
                        "w1a": w1a, "w0a": w0a})
    return in_maps


def _unshuffle_y(yd: np.ndarray) -> np.ndarray:
    """[i2a(8), (i0, i3b), (i3a, b, i1, i2b)] -> [N, BC]."""
    y = yd.astype(np.float32).reshape(8, 32, 4, 8, BC, 32, 4)
    y = y.transpose(1, 5, 0, 6, 3, 2, 4)
    return np.ascontiguousarray(y).reshape(N, BC)


def kernel(x, W0, W1, W2, W3, _trace=False):
    nc = _get_nc()
    in_maps = make_in_maps(x, W0, W1, W2, W3)
    res = run_bass_kernel_spmd(nc, in_maps, core_ids=list(range(NCORES)),
                               trace=_trace)
    out = np.concatenate(
        [_unshuffle_y(res.results[c]["y"]) for c in range(NCORES)], axis=1)
    if _trace:
        kernel.last_result = res
    return out


if __name__ == "__main__":
    rng = np.random.default_rng(0)
    x = rng.standard_normal((N, B), dtype=np.float32)
    ws = [rng.standard_normal((L, L), dtype=np.float32) for _ in range(4)]
    y = kernel(x, *ws)
    print("ran", y.shape, y.dtype)
